# revision 1
# baseline (speedup 1.0000x reference)
# Multi-head attention (B=4, L=2048, E=256, H=8) on 8 TRN2 NeuronCores.
#
# Sharding: core c handles batch b = c//2 and head group g = c%2 (heads
# 4g..4g+3).  Each core computes the partial output
#   sum_{h in group} softmax(x M_h x^T) (x N_h)
# for its batch, where the host pre-folds the per-head weights:
#   M_h = Wq_h Wk_h^T / sqrt(E)   (so scores = q k^T/16 = x M_h x^T)
#   N_h = Wv_h Wout_h             (so attn @ v @ Wout_h = (attn @ x) N_h)
# The host adds the two head-group partials per batch.  Folding removes
# the k and v projections from the device entirely.  The host also
# supplies x^T so the device does no transposes.
#
# Per-core dataflow (big matmuls in float32r, full PE rate at N>=256):
#   uT_h = M_h^T x^T                ([256, 2048], e2 on partitions)
#   per 512-wide qi block, streaming over 16 kj tiles:
#     sT   = xT[:,kj]^T uT   (PSUM [128kj, 512qi])   == scores^T
#     pT   = exp(sT)         (ACT, PSUM->SBUF)
#     colacc += pT           (DVE running sum for the softmax denominator)
#     wT  += x[kj]^T pT      (PSUM [128e, 512qi] = (p @ x)^T, acc over kj)
#   rowsum = colacc^T @ ones (PE, [128qi, 1]) ; recip = 1/rowsum (DVE)
#   out[qi] += (wT^T @ N_h) * recip   (fused scale+add on DVE)
# Scores never touch HBM; softmax normalization is applied after the
# output projection (row scaling commutes with right-multiplication).
# SBUF tiles feeding fp32r matmuls are declared float32r (the BIR
# verifier requires producers to round to fp32r); DVE/ACT consumers
# read them bitcast back to fp32.

import numpy as np

B, L, E, H = 4, 2048, 256, 8
HL = H // 2          # heads per core
LT = L // 128        # 16 row tiles
QB = 512             # qi block width
NQB = L // QB        # 4
KT = L // 128        # 16 kj tiles

_cache = {}


def _build_nc():
    import concourse.mybir as mybir
    from concourse import bacc
    from concourse.tile import TileContext

    F32 = mybir.dt.float32
    F32R = mybir.dt.float32r
    Exp = mybir.ActivationFunctionType.Exp

    def f(ap):  # read a float32r tile as plain f32 (same bits)
        return ap.bitcast(F32)

    nc = bacc.Bacc(None, target_bir_lowering=False)

    x_d = nc.dram_tensor("x", [L, E], F32, kind="ExternalInput")
    xt_d = nc.dram_tensor("xt", [E, L], F32, kind="ExternalInput")
    m_d = nc.dram_tensor("m", [E, HL * E], F32, kind="ExternalInput")
    n_d = nc.dram_tensor("n", [E, HL * E], F32, kind="ExternalInput")
    out_d = nc.dram_tensor("out", [L, E], F32, kind="ExternalOutput")

    with TileContext(nc) as tc:
        with (
            tc.tile_pool(name="const", bufs=1) as cpool,
            tc.tile_pool(name="head", bufs=2) as hpool,
            tc.tile_pool(name="work", bufs=2) as wpool,
            tc.tile_pool(name="ps_s", bufs=3, space="PSUM") as ps_s,
            tc.tile_pool(name="ps_ao", bufs=3, space="PSUM") as ps_ao,
            tc.tile_pool(name="ps_misc", bufs=2, space="PSUM") as ps_misc,
        ):
            ones = cpool.tile([128, 1], F32, name="ones")
            nc.vector.memset(ones, 1.0)

            # ---- x (natural + transposed, resident, float32r) ----
            xT = [cpool.tile([128, L], F32R, name=f"xT{i}") for i in range(2)]
            for i in range(2):
                for nb in range(NQB):
                    nc.sync.dma_start(
                        xT[i][:, nb * QB:(nb + 1) * QB],
                        xt_d[i * 128:(i + 1) * 128,
                             nb * QB:(nb + 1) * QB].bitcast(F32R))
            m_sb = [cpool.tile([128, HL * E], F32R, name=f"m{i}") for i in range(2)]
            for i in range(2):
                nc.sync.dma_start(m_sb[i], m_d[i * 128:(i + 1) * 128, :].bitcast(F32R))
            xn = [cpool.tile([128, E], F32R, name=f"xn{t}") for t in range(LT)]
            for t in range(LT):
                nc.sync.dma_start(xn[t], x_d[t * 128:(t + 1) * 128, :].bitcast(F32R))
            n_sb = [cpool.tile([128, HL * E], F32R, name=f"n{i}") for i in range(2)]
            for i in range(2):
                nc.sync.dma_start(n_sb[i], n_d[i * 128:(i + 1) * 128, :].bitcast(F32R))

            out_acc = [cpool.tile([128, E], F32, name=f"oacc{t}") for t in range(LT)]

            for h in range(HL):
                # ---- uT_h = M_h^T x^T  ([256, 2048] as 2 e2-halves) ----
                uT = [hpool.tile([128, L], F32R, name=f"uT{eh}", tag=f"uT{eh}")
                      for eh in range(2)]
                for eh in range(2):
                    for nb in range(NQB):
                        ps = ps_s.tile([128, QB], F32, name="ups", tag="s")
                        for ih in range(2):
                            nc.tensor.matmul(
                                ps,
                                m_sb[ih][:, h * E + eh * 128:h * E + (eh + 1) * 128],
                                xT[ih][:, nb * QB:(nb + 1) * QB],
                                start=(ih == 0), stop=(ih == 1),
                            )
                        nc.vector.tensor_copy(uT[eh][:, nb * QB:(nb + 1) * QB], ps)

                # ---- attention, one 512-wide qi block at a time ----
                for qb in range(NQB):
                    colacc = wpool.tile([128, QB], F32, name="colacc", tag="colacc")
                    w_ps = [ps_ao.tile([128, QB], F32, name=f"wps{eh}", tag="ao")
                            for eh in range(2)]
                    for t in range(KT):
                        s_ps = ps_s.tile([128, QB], F32, name="sps", tag="s")
                        for eh in range(2):
                            nc.tensor.matmul(
                                s_ps,
                                xT[eh][:, t * 128:(t + 1) * 128],
                                uT[eh][:, qb * QB:(qb + 1) * QB],
                                start=(eh == 0), stop=(eh == 1),
                            )
                        pt = wpool.tile([128, QB], F32R, name="pt", tag="pt", bufs=4)
                        nc.scalar.activation(pt, s_ps, Exp)
                        if t == 0:
                            nc.vector.tensor_copy(colacc, f(pt))
                        else:
                            nc.vector.tensor_add(colacc, colacc, f(pt))
                        for eh in range(2):
                            nc.tensor.matmul(
                                w_ps[eh],
                                xn[t][:, eh * 128:(eh + 1) * 128],
                                pt,
                                start=(t == 0), stop=(t == KT - 1),
                            )
                    wT = [wpool.tile([128, QB], F32R, name=f"wT{eh}", tag=f"wT{eh}")
                          for eh in range(2)]
                    for eh in range(2):
                        nc.vector.tensor_copy(wT[eh], w_ps[eh])
                    for j in range(QB // 128):
                        rs_ps = ps_s.tile([128, 1], F32, name="rsps", tag="s")
                        nc.tensor.matmul(rs_ps, colacc[:, j * 128:(j + 1) * 128],
                                         ones, start=True, stop=True)
                        recip = wpool.tile([128, 1], F32, name="recip", tag="recip",
                                           bufs=4)
                        nc.vector.reciprocal(recip, rs_ps)
                        pj_ps = ps_misc.tile([128, E], F32, name="pjps", tag="misc")
                        for eh in range(2):
                            nc.tensor.matmul(
                                pj_ps,
                                wT[eh][:, j * 128:(j + 1) * 128],
                                n_sb[eh][:, h * E:(h + 1) * E],
                                start=(eh == 0), stop=(eh == 1),
                            )
                        gt = qb * (QB // 128) + j
                        if h == 0:
                            nc.vector.tensor_scalar_mul(out_acc[gt], pj_ps, recip)
                        else:
                            nc.vector.scalar_tensor_tensor(
                                out_acc[gt], pj_ps, recip, out_acc[gt],
                                op0=mybir.AluOpType.mult, op1=mybir.AluOpType.add)

            for t in range(LT):
                nc.sync.dma_start(out_d[t * 128:(t + 1) * 128, :], out_acc[t])

    nc.compile()
    return nc


def _get_nc():
    if "nc" not in _cache:
        _cache["nc"] = _build_nc()
    return _cache["nc"]


def _in_maps(x, W_qkv, W_out):
    x = np.ascontiguousarray(np.asarray(x, dtype=np.float32))
    W_qkv = np.asarray(W_qkv, dtype=np.float32)
    W_out = np.asarray(W_out, dtype=np.float32)

    # Host-side weight folding (float64 for exactness, cast to f32):
    #   M_h = Wq_h Wk_h^T / sqrt(E),   N_h = Wv_h Wout_h
    Wq = W_qkv[:, 0:H * E].astype(np.float64)
    Wk = W_qkv[:, H * E:2 * H * E].astype(np.float64)
    Wv = W_qkv[:, 2 * H * E:3 * H * E].astype(np.float64)
    Wo = W_out.astype(np.float64)
    scale = 1.0 / np.sqrt(E)
    M = np.empty((H, E, E), np.float64)
    N = np.empty((H, E, E), np.float64)
    for h in range(H):
        M[h] = (Wq[:, h * E:(h + 1) * E] @ Wk[:, h * E:(h + 1) * E].T) * scale
        N[h] = Wv[:, h * E:(h + 1) * E] @ Wo[h * E:(h + 1) * E, :]

    maps = []
    for c in range(2 * B):
        b, g = c // 2, c % 2
        hs = HL * g  # first head of this core's group
        mcat = np.concatenate([M[hs + i] for i in range(HL)], axis=1)
        ncat = np.concatenate([N[hs + i] for i in range(HL)], axis=1)
        maps.append({
            "x": np.ascontiguousarray(x[b]),
            "xt": np.ascontiguousarray(x[b].T),
            "m": np.ascontiguousarray(mcat.astype(np.float32)),
            "n": np.ascontiguousarray(ncat.astype(np.float32)),
        })
    return maps


def kernel(x, W_qkv, W_out, _trace=False):
    from concourse.bass_utils import run_bass_kernel_spmd

    nc = _get_nc()
    maps = _in_maps(x, W_qkv, W_out)
    res = run_bass_kernel_spmd(nc, maps, core_ids=list(range(2 * B)),
                               trace=_trace)
    _cache["last_result"] = res
    outs = [m["out"] for m in res.results]
    full = np.stack([outs[2 * b] + outs[2 * b + 1] for b in range(B)])
    return full.astype(np.float32)



# revision 6
# speedup vs baseline: 1.3662x; 1.3662x over previous
# Multi-head attention (B=4, L=2048, E=256, H=8) on 8 TRN2 NeuronCores.
#
# Sharding: core c handles batch b = c//2 and head group g = c%2 (heads
# 4g..4g+3).  Each core computes the partial output
#   sum_{h in group} softmax(x M_h x^T) (x N_h)
# for its batch, with host-folded per-head weights:
#   M_h = Wq_h Wk_h^T / sqrt(E)   (so scores = x M_h x^T)
#   N_h = Wv_h Wout_h             (so attn @ v @ Wout_h = (attn @ x) N_h)
# The host adds the two head-group partials per batch.
#
# Precision strategy (rel-err budget is 2e-2; this lands ~5e-3):
#   - scores matmuls (uT = M^T x^T and sT = xT^T uT) run in fp8 e4m3 with
#     perf_mode=DoubleRow: contraction of 256 packed as [128, 2, N] k-tile
#     pairs, one PE pass instead of two (~1.8x on the scores GEMMs).
#     M is pre-scaled by 1024 on the host so u-values sit in e4m3's
#     normal range; exp() unscales via its free `scale` operand.
#   - p = exp(s), wT = x^T p, and the out-projection run in bf16
#     (bf16 matmul = fp32r rate, but enables FWL weight loads and 1024-wide
#     moving operands).
#   - PSUM accumulation is fp32 throughout; softmax normalization (row
#     scaling) is applied after the output projection where it commutes.
#
# Per-core dataflow, per (head, 1024-wide qi block):
#   for each of 16 kj tiles: sT = DR-matmul(xT8_kj, uT8_qb)  (PSUM [128,1024])
#     pt = exp(sT/1024) -> bf16 SBUF (ACT, one 1024-wide op per tile)
#     w_psA += xn_kj^T pt[:, 0:512]  (both e-halves into one 2-bank tile)
#     bf16 pair-tree on DVE accumulates colacc = sum_kj pt
#   wTA = cast(w_psA); second pass over pt for qi 512:1024 -> wTB
#   rowsum via 8 tiny matmuls (colacc chunks ^T @ ones), one reciprocal
#   pj = wT^T N_h (PSUM), out_acc += pj * recip  (DVE scalar_tensor_tensor)
# Scores never touch HBM.

import numpy as np

B, L, E, H = 4, 2048, 256, 8
HL = H // 2          # heads per core
QB = 1024            # qi block width
NQB = L // QB        # 2
KT = L // 128        # 16 kj tiles
HE = H * E

_cache = {}


def _build_nc():
    import concourse.mybir as mybir
    from concourse import bacc
    from concourse.tile import TileContext

    F32 = mybir.dt.float32
    BF16 = mybir.dt.bfloat16
    F8 = mybir.dt.float8e4
    Exp = mybir.ActivationFunctionType.Exp
    DR = mybir.MatmulPerfMode.DoubleRow

    nc = bacc.Bacc(None, target_bir_lowering=False)

    x_d = nc.dram_tensor("x", [L, E], BF16, kind="ExternalInput")
    xt8_d = nc.dram_tensor("xt8", [128, 2, L], F8, kind="ExternalInput")
    m8_d = nc.dram_tensor("m8", [128, 2, HL * E], F8, kind="ExternalInput")
    n_d = nc.dram_tensor("n", [128, 2, HL * E], BF16, kind="ExternalInput")
    out_d = nc.dram_tensor("out", [L, E], F32, kind="ExternalOutput")

    with TileContext(nc) as tc:
        with (
            tc.tile_pool(name="const", bufs=1) as cpool,
            tc.tile_pool(name="head", bufs=2) as hpool,
            tc.tile_pool(name="pt", bufs=18) as ptpool,
            tc.tile_pool(name="l1", bufs=5) as l1pool,
            tc.tile_pool(name="l2", bufs=3) as l2pool,
            tc.tile_pool(name="l3", bufs=2) as l3pool,
            tc.tile_pool(name="cacc", bufs=2) as capool,
            tc.tile_pool(name="wt", bufs=3) as wtpool,
            tc.tile_pool(name="rc", bufs=2) as rcpool,
            tc.tile_pool(name="ps_s", bufs=2, space="PSUM") as ps_s,
            tc.tile_pool(name="ps_w", bufs=1, space="PSUM") as ps_w,
            tc.tile_pool(name="ps_pj", bufs=2, space="PSUM") as ps_pj,
        ):
            ones = cpool.tile([128, 1], BF16, name="ones")
            nc.vector.memset(ones, 1.0)

            # ---- resident inputs ----
            m8 = cpool.tile([128, 2, HL * E], F8, name="m8")
            nc.sync.dma_start(m8, m8_d[:, :, :])
            xt8 = cpool.tile([128, 2, L], F8, name="xt8")
            for nb in range(NQB):
                nc.sync.dma_start(xt8[:, :, nb * QB:(nb + 1) * QB],
                                  xt8_d[:, :, nb * QB:(nb + 1) * QB])
            xn = [cpool.tile([128, E], BF16, name=f"xn{t}") for t in range(KT)]
            for t in range(KT):
                nc.sync.dma_start(xn[t], x_d[t * 128:(t + 1) * 128, :])
            nsb = cpool.tile([128, 2, HL * E], BF16, name="nsb")
            nc.sync.dma_start(nsb, n_d[:, :, :])

            out_acc = [cpool.tile([128, E], F32, name=f"oacc{t}")
                       for t in range(KT)]

            for h in range(HL):
                # ---- uT8 = (1024 * M_h)^T x^T, fp8-packed [128, 2, L] ----
                uT8 = hpool.tile([128, 2, L], F8, name="uT8", tag="uT8")
                for eh in range(2):
                    for nb in range(NQB):
                        u_ps = ps_s.tile([128, QB], F32, name="ups", tag="s")
                        for sh in range(2):
                            nc.tensor.matmul(
                                u_ps[:, sh * 512:(sh + 1) * 512],
                                m8[:, :,
                                   h * E + eh * 128:h * E + (eh + 1) * 128],
                                xt8[:, :,
                                    nb * QB + sh * 512:nb * QB + (sh + 1) * 512],
                                start=True, stop=True, perf_mode=DR,
                            )
                        nc.vector.tensor_copy(
                            uT8[:, eh, nb * QB:(nb + 1) * QB], u_ps)

                for qb in range(NQB):
                    pts = []
                    l1 = []
                    w_ps = ps_w.tile([128, QB], F32, name="wpsA", tag="w")
                    for t in range(KT):
                        s_ps = ps_s.tile([128, QB], F32, name="sps", tag="s")
                        for sh in range(2):
                            nc.tensor.matmul(
                                s_ps[:, sh * 512:(sh + 1) * 512],
                                xt8[:, :, t * 128:(t + 1) * 128],
                                uT8[:, :,
                                    qb * QB + sh * 512:qb * QB + (sh + 1) * 512],
                                start=True, stop=True, perf_mode=DR,
                            )
                        pt = ptpool.tile([128, QB], BF16, name="pt", tag="pt")
                        nc.scalar.activation(pt, s_ps, Exp, scale=1.0 / 1024.0)
                        pts.append(pt)
                        for eh in range(2):
                            nc.tensor.matmul(
                                w_ps[:, eh * 512:(eh + 1) * 512],
                                xn[t][:, eh * 128:(eh + 1) * 128],
                                pt[:, 0:512],
                                start=(t == 0), stop=(t == KT - 1),
                            )
                        # bf16 pair tree, level 1 (defer the last pair so the
                        # wTA cast reaches the DVE queue first)
                        if t % 2 == 1 and t < KT - 1:
                            s1 = l1pool.tile([128, QB], BF16, name="s1",
                                             tag="l1")
                            nc.vector.tensor_add(s1, pts[t - 1], pts[t])
                            l1.append(s1)
                    wTA = wtpool.tile([128, QB], BF16, name="wTA", tag="wt")
                    nc.vector.tensor_copy(wTA, w_ps)
                    # finish the reduction tree
                    s1 = l1pool.tile([128, QB], BF16, name="s1", tag="l1")
                    nc.vector.tensor_add(s1, pts[KT - 2], pts[KT - 1])
                    l1.append(s1)
                    l2 = []
                    for i in range(4):
                        s2 = l2pool.tile([128, QB], BF16, name="s2", tag="l2")
                        nc.vector.tensor_add(s2, l1[2 * i], l1[2 * i + 1])
                        l2.append(s2)
                    l3 = []
                    for i in range(2):
                        s3 = l3pool.tile([128, QB], BF16, name="s3", tag="l3")
                        nc.vector.tensor_add(s3, l2[2 * i], l2[2 * i + 1])
                        l3.append(s3)
                    colacc = capool.tile([128, QB], BF16, name="colacc",
                                         tag="cacc")
                    nc.vector.tensor_add(colacc, l3[0], l3[1])

                    # second qi-half pass over the same pt tiles
                    w_ps = ps_w.tile([128, QB], F32, name="wpsB", tag="w")
                    for t in range(KT):
                        for eh in range(2):
                            nc.tensor.matmul(
                                w_ps[:, eh * 512:(eh + 1) * 512],
                                xn[t][:, eh * 128:(eh + 1) * 128],
                                pts[t][:, 512:1024],
                                start=(t == 0), stop=(t == KT - 1),
                            )
                    wTB = wtpool.tile([128, QB], BF16, name="wTB", tag="wt")
                    nc.vector.tensor_copy(wTB, w_ps)

                    # softmax denominators for the 8 qi chunks of this block
                    rs = ps_pj.tile([128, 8], F32, name="rs", tag="pj")
                    for j in range(8):
                        nc.tensor.matmul(rs[:, j:j + 1],
                                         colacc[:, j * 128:(j + 1) * 128],
                                         ones, start=True, stop=True)
                    recip = rcpool.tile([128, 8], F32, name="recip", tag="rc")
                    nc.vector.reciprocal(recip, rs)

                    for j in range(8):
                        wT = wTA if j < 4 else wTB
                        jj = j % 4
                        pj = ps_pj.tile([128, E], F32, name="pj", tag="pj")
                        for eh in range(2):
                            nc.tensor.matmul(
                                pj,
                                wT[:, eh * 512 + jj * 128:
                                   eh * 512 + (jj + 1) * 128],
                                nsb[:, eh, h * E:(h + 1) * E],
                                start=(eh == 0), stop=(eh == 1),
                            )
                        gt = qb * 8 + j
                        if h == 0:
                            nc.vector.tensor_scalar_mul(
                                out_acc[gt], pj, recip[:, j:j + 1])
                        else:
                            nc.vector.scalar_tensor_tensor(
                                out_acc[gt], pj, recip[:, j:j + 1],
                                out_acc[gt],
                                op0=mybir.AluOpType.mult,
                                op1=mybir.AluOpType.add)
                        if h == HL - 1:
                            nc.sync.dma_start(
                                out_d[gt * 128:(gt + 1) * 128, :],
                                out_acc[gt])

    nc.compile()
    return nc


def _get_nc():
    if "nc" not in _cache:
        _cache["nc"] = _build_nc()
    return _cache["nc"]


def _in_maps(x, W_qkv, W_out):
    import ml_dtypes

    f8 = ml_dtypes.float8_e4m3
    bf16 = ml_dtypes.bfloat16

    x = np.ascontiguousarray(np.asarray(x, dtype=np.float32))
    W_qkv = np.asarray(W_qkv, dtype=np.float32)
    W_out = np.asarray(W_out, dtype=np.float32)

    # Host-side weight folding (float64 for exactness):
    #   M_h = Wq_h Wk_h^T / sqrt(E) * 1024  (fp8 range lift),  N_h = Wv_h Wout_h
    Wq = W_qkv[:, 0:HE].astype(np.float64)
    Wk = W_qkv[:, HE:2 * HE].astype(np.float64)
    Wv = W_qkv[:, 2 * HE:3 * HE].astype(np.float64)
    Wo = W_out.astype(np.float64)
    scale = 1024.0 / np.sqrt(E)
    M = np.empty((H, E, E), np.float64)
    N = np.empty((H, E, E), np.float64)
    for h in range(H):
        M[h] = (Wq[:, h * E:(h + 1) * E] @ Wk[:, h * E:(h + 1) * E].T) * scale
        N[h] = Wv[:, h * E:(h + 1) * E] @ Wo[h * E:(h + 1) * E, :]

    maps = []
    for c in range(2 * B):
        b, g = c // 2, c % 2
        hs = HL * g
        mcat = np.concatenate([M[hs + i] for i in range(HL)], axis=1)
        ncat = np.concatenate([N[hs + i] for i in range(HL)], axis=1)
        xb = x[b]
        # [128, 2, X] k-tile-pair layout: element (i, j, c) = src[128*j + i, c]
        xt8 = np.ascontiguousarray(
            xb.T.reshape(2, 128, L).transpose(1, 0, 2)).astype(f8)
        m8 = np.ascontiguousarray(
            mcat.reshape(2, 128, HL * E).transpose(1, 0, 2)).astype(f8)
        n8 = np.ascontiguousarray(
            ncat.reshape(2, 128, HL * E).transpose(1, 0, 2)).astype(bf16)
        maps.append({
            "x": xb.astype(bf16),
            "xt8": xt8,
            "m8": m8,
            "n": n8,
        })
    return maps


def kernel(x, W_qkv, W_out, _trace=False):
    from concourse.bass_utils import run_bass_kernel_spmd

    nc = _get_nc()
    maps = _in_maps(x, W_qkv, W_out)
    res = run_bass_kernel_spmd(nc, maps, core_ids=list(range(2 * B)),
                               trace=_trace)
    _cache["last_result"] = res
    outs = [m["out"] for m in res.results]
    full = np.stack([outs[2 * b] + outs[2 * b + 1] for b in range(B)])
    return full.astype(np.float32)
